# revision 8
# baseline (speedup 1.0000x reference)
"""Trainium2 Bass kernel for nn_Net_14869176779172 (moe_routing).

Computes, for x[B=1024, D=4096, S=60], W[D, S], soma_w[D], soma_b[1]:
    d[b, j]  = sum_s x[b, j, s] * W[j, s]          (per-dendrite dot)
    r        = relu(d)
    act[b,j] = sigmoid(r)        for j < 1638      (first 40% of dendrites)
             = sqrt(r)           otherwise
    out[b]   = act[b, :] @ soma_w + soma_b         -> [B, 1]

Sharding: pure data-parallel over batch across 8 NeuronCores (128 batch
rows per core); W / soma_w / soma_b replicated.

Per-core layout (v2): partition p = jblk*4 + bidx, where jblk in [0,32)
indexes a block of 128 dendrites and bidx in [0,4) a batch row within a
4-row group.  W is replicated only 4x (host-prepped [128, 7680] fp16
tile, resident in SBUF all kernel), so there is NO per-chunk W broadcast
machinery.  x streams in 32 cast-DMAs (fp32->fp16, 3.9 MB each, 30 KB
contiguous per partition line); DVE does an in-place multiply (2x mode)
+ segmented reduce per group.  Both sigmoid and sqrt are applied
full-width on ScalarE; region selection is folded into two masked
soma-weight tensors (avoids partition-sliced ops, which the BIR
verifier rejects off 32-partition boundaries).  The final
cross-partition soma sum (over jblk) is one matmul with a selector.
"""

import numpy as np

import concourse.bacc as bacc
import concourse.bass as bass
import concourse.tile as tile
from concourse import mybir
from concourse.bass_utils import run_bass_kernel_spmd

# Problem constants (hardcoded per harness contract).
B_TOTAL = 1024
N_CORES = 8
B = B_TOTAL // N_CORES  # 128 batch rows per core
D = 4096
S = 60
CUT = int(D * 0.4)  # 1638: first CUT dendrites use sigmoid, rest sqrt

P = 128  # SBUF partitions
NJ = 32  # dendrite blocks
DJ = D // NJ  # 128 dendrites per block
NB = 4  # batch rows per group (per DMA)
NG = B // NB  # 32 groups
GF = DJ * S  # 7680 elems per partition per group
ROW_F = D * S  # 245760 elems per batch row

FP32 = mybir.dt.float32
FP16 = mybir.dt.float16


def _build_program():
    nc = bacc.Bacc(
        "TRN2",
        target_bir_lowering=False,
        debug=False,
        enable_asserts=False,
        num_devices=N_CORES,
    )

    x_d = nc.dram_tensor("x", [B, D, S], FP32, kind="ExternalInput")
    w2_d = nc.dram_tensor("W2", [P, GF], FP16, kind="ExternalInput")
    # packed small inputs (the NEFF loader rejects >5 input tensors):
    # SWPAIR = [sw_sig | sw_sqrt] fp16; SELSB = [sel | soma_b] fp32
    swpair_d = nc.dram_tensor("SWPAIR", [P, 2 * DJ], FP16, kind="ExternalInput")
    selsb_d = nc.dram_tensor("SELSB", [P, NB + 1], FP32, kind="ExternalInput")
    out_d = nc.dram_tensor("out", [B, 1], FP32, kind="ExternalOutput")

    x_ap = x_d.ap().rearrange("b d s -> b (d s)")  # [128, 245760]

    with tile.TileContext(nc) as tc:
        with (
            tc.tile_pool(name="singles", bufs=1) as singles,
            tc.tile_pool(name="xpool", bufs=6) as xpool,
            tc.tile_pool(name="zpool", bufs=3) as zpool,
            tc.tile_pool(name="zspool", bufs=3) as zspool,
            tc.tile_pool(name="scrpool", bufs=2) as scrpool,
            tc.tile_pool(name="psum", bufs=1, space="PSUM") as psum_pool,
        ):
            # ---- resident small tensors (HWDGE loads, overlap with x DMA) ----
            w2 = singles.tile([P, GF], FP16)
            nc.sync.dma_start(out=w2, in_=w2_d.ap())
            sw_pair = singles.tile([P, 2 * DJ], FP16)
            nc.sync.dma_start(out=sw_pair, in_=swpair_d.ap())
            sw_sig = sw_pair[:, 0:DJ]
            sw_sqrt = sw_pair[:, DJ : 2 * DJ]
            selsb = singles.tile([P, NB + 1], FP32)
            nc.sync.dma_start(out=selsb, in_=selsb_d.ap())
            sel = selsb[:, 0:NB]
            sbb = selsb[:, NB : NB + 1]

            # per-(partition, group) soma partial sums: [:, g] sigmoid-region,
            # [:, NG+g] sqrt-region
            acc = singles.tile([P, 2 * NG], FP32)

            for g in range(NG):
                # x group: partition p=(jblk,bidx) reads batch row g*NB+bidx,
                # dendrites [jblk*DJ, (jblk+1)*DJ) -- 30 KB contiguous fp32,
                # cast to fp16 in the SDMA datapath (SWDGE).
                xt = xpool.tile([P, GF], FP16)
                src = bass.AP(
                    tensor=x_ap.tensor,
                    offset=x_ap.offset + g * NB * ROW_F,
                    ap=[[GF, NJ], [ROW_F, NB], [1, GF]],
                )
                nc.gpsimd.dma_start(out=xt, in_=src)

                # y = x * W (in-place, fp16 2x mode)
                nc.vector.tensor_mul(xt, xt, w2)

                # s-reduction: tensor_reduce has no 2x uop (runs 1x), so do
                # two halving tensor_adds first (GpSimd; 2x-capable shapes)
                # and only reduce the last 15 on DVE.
                xt3 = xt.rearrange("p (c s) -> p c s", s=S)
                nc.gpsimd.tensor_add(
                    xt3[:, :, 0:30], xt3[:, :, 0:30], xt3[:, :, 30:60]
                )
                nc.gpsimd.tensor_add(
                    xt3[:, :, 0:15], xt3[:, :, 0:15], xt3[:, :, 15:30]
                )
                zg = zpool.tile([P, DJ], FP16)
                with nc.allow_low_precision(
                    "fp16 dendrite sums; tol 2e-2, DVE accumulates fp32 internally"
                ):
                    nc.vector.tensor_reduce(
                        out=zg,
                        in_=xt3[:, :, 0:15],
                        axis=mybir.AxisListType.X,
                        op=mybir.AluOpType.add,
                    )

                # r = relu(z); zs = sigmoid(r); zg <- sqrt(r)   (all ScalarE)
                nc.scalar.activation(
                    out=zg, in_=zg, func=mybir.ActivationFunctionType.Relu
                )
                zs = zspool.tile([P, DJ], FP16)
                nc.scalar.activation(
                    out=zs, in_=zg, func=mybir.ActivationFunctionType.Sigmoid
                )
                nc.scalar.activation(
                    out=zg, in_=zg, func=mybir.ActivationFunctionType.Sqrt
                )

                # acc[p, g] = sum_c zs*sw_sig;  acc[p, NG+g] = sum_c zq*sw_sqrt
                # (tensor_tensor_reduce crashes the runtime; use mul+reduce)
                scr = scrpool.tile([P, DJ], FP16)
                nc.vector.tensor_mul(scr, zs, sw_sig)
                nc.vector.tensor_reduce(
                    out=acc[:, g : g + 1],
                    in_=scr,
                    axis=mybir.AxisListType.X,
                    op=mybir.AluOpType.add,
                )
                scr2 = scrpool.tile([P, DJ], FP16)
                nc.vector.tensor_mul(scr2, zg, sw_sqrt)
                nc.vector.tensor_reduce(
                    out=acc[:, NG + g : NG + g + 1],
                    in_=scr2,
                    axis=mybir.AxisListType.X,
                    op=mybir.AluOpType.add,
                )

            # ---- final: out[g*NB+i] = sum_jblk (acc_sig + acc_sqrt) + soma_b
            ps = psum_pool.tile([NB, 2 * NG], FP32)
            nc.tensor.matmul(ps, sel, acc)
            ps_sb = singles.tile([NB, 2 * NG], FP32)
            nc.scalar.copy(out=ps_sb, in_=ps)
            out_sb = singles.tile([NB, NG], FP32)
            nc.vector.tensor_add(out_sb, ps_sb[:, 0:NG], ps_sb[:, NG : 2 * NG])
            nc.vector.tensor_scalar_add(out=out_sb, in0=out_sb, scalar1=sbb[0:NB])
            oa = out_d.ap().rearrange("b one -> (b one)")
            nc.sync.dma_start(
                out=bass.AP(tensor=oa.tensor, offset=oa.offset, ap=[[1, NB], [NB, NG]]),
                in_=out_sb,
            )

    nc.compile()
    return nc


_NC_CACHE = None


def _get_program():
    global _NC_CACHE
    if _NC_CACHE is None:
        _NC_CACHE = _build_program()
    return _NC_CACHE


def kernel(x, W, soma_w, soma_b, _trace=False):
    nc = _get_program()
    x = np.ascontiguousarray(x, dtype=np.float32)
    W16 = np.asarray(W, dtype=np.float16)
    soma_w = np.asarray(soma_w, dtype=np.float32)
    soma_b = np.asarray(soma_b, dtype=np.float32)

    w2 = np.ascontiguousarray(np.repeat(W16.reshape(NJ, GF), NB, axis=0))
    sw16 = soma_w.astype(np.float16)
    is_sig = np.arange(D) < CUT
    sw_sig = np.repeat(np.where(is_sig, sw16, 0).reshape(NJ, DJ), NB, axis=0)
    sw_sqrt = np.repeat(np.where(is_sig, 0, sw16).reshape(NJ, DJ), NB, axis=0)
    sw_pair = np.ascontiguousarray(np.concatenate([sw_sig, sw_sqrt], axis=1))
    sel = (np.arange(P)[:, None] % NB == np.arange(NB)[None, :]).astype(np.float32)
    sb = np.full((P, 1), float(soma_b.reshape(-1)[0]), np.float32)
    selsb = np.ascontiguousarray(np.concatenate([sel, sb], axis=1))

    in_maps = [
        {
            "x": x[i * B : (i + 1) * B],
            "W2": w2,
            "SWPAIR": sw_pair,
            "SELSB": selsb,
        }
        for i in range(N_CORES)
    ]
    res = run_bass_kernel_spmd(
        nc, in_maps, core_ids=list(range(N_CORES)), trace=_trace
    )
    out = np.concatenate([r["out"] for r in res.results], axis=0)
    if _trace:
        kernel.last_results = res
    return out.astype(np.float32)


# revision 11
# speedup vs baseline: 1.4171x; 1.4171x over previous
"""Trainium2 Bass kernel for nn_Net_14869176779172 (moe_routing).

Computes, for x[B=1024, D=4096, S=60], W[D, S], soma_w[D], soma_b[1]:
    d[b, j]  = sum_s x[b, j, s] * W[j, s]          (per-dendrite dot)
    r        = relu(d)
    act[b,j] = sigmoid(r)        for j < 1638      (first 40% of dendrites)
             = sqrt(r)           otherwise
    out[b]   = act[b, :] @ soma_w + soma_b         -> [B, 1]

Sharding: pure data-parallel over batch across 8 NeuronCores (128 batch
rows per core); W / soma_w / soma_b replicated.

Per-core layout (v2): partition p = jblk*4 + bidx, where jblk in [0,32)
indexes a block of 128 dendrites and bidx in [0,4) a batch row within a
4-row group.  W is replicated only 4x (host-prepped [128, 7680] fp16
tile, resident in SBUF all kernel), so there is NO per-chunk W broadcast
machinery.  x streams in 32 cast-DMAs (fp32->fp16, 3.9 MB each, 30 KB
contiguous per partition line); DVE does an in-place multiply (2x mode)
+ segmented reduce per group.  Both sigmoid and sqrt are applied
full-width on ScalarE; region selection is folded into two masked
soma-weight tensors (avoids partition-sliced ops, which the BIR
verifier rejects off 32-partition boundaries).  The final
cross-partition soma sum (over jblk) is one matmul with a selector.
"""

import numpy as np

import concourse.bacc as bacc
import concourse.bass as bass
import concourse.tile as tile
from concourse import mybir
from concourse.bass_utils import run_bass_kernel_spmd

# Problem constants (hardcoded per harness contract).
B_TOTAL = 1024
N_CORES = 8
B = B_TOTAL // N_CORES  # 128 batch rows per core
D = 4096
S = 60
CUT = int(D * 0.4)  # 1638: first CUT dendrites use sigmoid, rest sqrt

P = 128  # SBUF partitions
NJ = 32  # dendrite blocks
DJ = D // NJ  # 128 dendrites per block
NB = 4  # batch rows per group (per DMA)
NG = B // NB  # 32 groups
GF = DJ * S  # 7680 elems per partition per group
ROW_F = D * S  # 245760 elems per batch row

FP32 = mybir.dt.float32
FP16 = mybir.dt.float16


def _build_program():
    nc = bacc.Bacc(
        "TRN2",
        target_bir_lowering=False,
        debug=False,
        enable_asserts=False,
        num_devices=N_CORES,
    )

    x_d = nc.dram_tensor("x", [B, D, S], FP32, kind="ExternalInput")
    w2_d = nc.dram_tensor("W2", [P, GF], FP16, kind="ExternalInput")
    # packed small inputs (the NEFF loader rejects >5 input tensors):
    # SWPAIR = [sw_sig | sw_sqrt] fp16; SELSB = [sel | soma_b] fp32
    swpair_d = nc.dram_tensor("SWPAIR", [P, 2 * DJ], FP16, kind="ExternalInput")
    selsb_d = nc.dram_tensor("SELSB", [P, NB + 1], FP32, kind="ExternalInput")
    out_d = nc.dram_tensor("out", [B, 1], FP32, kind="ExternalOutput")

    x_ap = x_d.ap().rearrange("b d s -> b (d s)")  # [128, 245760]

    with tile.TileContext(nc) as tc:
        with (
            tc.tile_pool(name="singles", bufs=1) as singles,
            tc.tile_pool(name="xpool", bufs=6) as xpool,
            tc.tile_pool(name="y2pool", bufs=3) as y2pool,
            tc.tile_pool(name="y3pool", bufs=3) as y3pool,
            tc.tile_pool(name="zpool", bufs=3) as zpool,
            tc.tile_pool(name="zzpool", bufs=3) as zzpool,
            tc.tile_pool(name="scrpool", bufs=2) as scrpool,
            tc.tile_pool(name="psum", bufs=1, space="PSUM") as psum_pool,
        ):
            # ---- resident small tensors (HWDGE loads, overlap with x DMA) ----
            w2 = singles.tile([P, GF], FP16)
            nc.sync.dma_start(out=w2, in_=w2_d.ap())
            sw_pair = singles.tile([P, 2 * DJ], FP16)
            nc.sync.dma_start(out=sw_pair, in_=swpair_d.ap())
            sw_sig = sw_pair[:, 0:DJ]
            sw_sqrt = sw_pair[:, DJ : 2 * DJ]
            selsb = singles.tile([P, NB + 1], FP32)
            nc.sync.dma_start(out=selsb, in_=selsb_d.ap())
            sel = selsb[:, 0:NB]
            sbb = selsb[:, NB : NB + 1]

            # per-(partition, group) soma partial sums: [:, g] sigmoid-region,
            # [:, NG+g] sqrt-region
            acc = singles.tile([P, 2 * NG], FP32)

            for g in range(NG):
                # x group: partition p=(jblk,bidx) reads batch row g*NB+bidx,
                # dendrites [jblk*DJ, (jblk+1)*DJ) -- 30 KB contiguous fp32,
                # cast to fp16 in the SDMA datapath (SWDGE).
                xt = xpool.tile([P, GF], FP16)
                src = bass.AP(
                    tensor=x_ap.tensor,
                    offset=x_ap.offset + g * NB * ROW_F,
                    ap=[[GF, NJ], [ROW_F, NB], [1, GF]],
                )
                nc.gpsimd.dma_start(out=xt, in_=src)

                # y = x * W (in-place, fp16 2x mode)
                nc.vector.tensor_mul(xt, xt, w2)

                # s-reduction 60 -> 30 -> 15 via dense-output tensor_adds
                # (fp16 2x on DVE; strided-OUT ops are slow everywhere, and
                # tensor_reduce has no 2x uop, so reduce only the last 15).
                # Alternate the halving engine per group parity to split the
                # load between DVE and GpSimd.
                xt3 = xt.rearrange("p (c s) -> p c s", s=S)
                y2 = y2pool.tile([P, DJ * 30], FP16)
                y23 = y2.rearrange("p (c s) -> p c s", s=30)
                y3 = y3pool.tile([P, DJ * 15], FP16)
                y33 = y3.rearrange("p (c s) -> p c s", s=15)
                half_eng = nc.vector if g % 2 == 0 else nc.gpsimd
                half_eng.tensor_add(y23, xt3[:, :, 0:30], xt3[:, :, 30:60])
                half_eng.tensor_add(y33, y23[:, :, 0:15], y23[:, :, 15:30])
                zg = zpool.tile([P, DJ], FP16)
                with nc.allow_low_precision(
                    "fp16 dendrite sums; tol 2e-2, DVE accumulates fp32 internally"
                ):
                    nc.vector.tensor_reduce(
                        out=zg,
                        in_=y33,
                        axis=mybir.AxisListType.X,
                        op=mybir.AluOpType.add,
                    )

                # r = relu(z); zz = [sigmoid(r) | sqrt(r)]   (all ScalarE)
                nc.scalar.activation(
                    out=zg, in_=zg, func=mybir.ActivationFunctionType.Relu
                )
                zz = zzpool.tile([P, 2 * DJ], FP16)
                nc.scalar.activation(
                    out=zz[:, 0:DJ], in_=zg, func=mybir.ActivationFunctionType.Sigmoid
                )
                nc.scalar.activation(
                    out=zz[:, DJ : 2 * DJ],
                    in_=zg,
                    func=mybir.ActivationFunctionType.Sqrt,
                )

                # acc[p, 2g+r] = sum_c zz[r]*sw_pair[r]  (r=0 sig, r=1 sqrt)
                # (tensor_tensor_reduce crashes the runtime; use mul+reduce)
                scr = scrpool.tile([P, 2 * DJ], FP16)
                nc.vector.tensor_mul(scr, zz, sw_pair)
                nc.vector.tensor_reduce(
                    out=acc[:, 2 * g : 2 * g + 2],
                    in_=scr.rearrange("p (r c) -> p r c", r=2),
                    axis=mybir.AxisListType.X,
                    op=mybir.AluOpType.add,
                )

            # ---- final: out[g*NB+i] = sum_jblk (acc_sig + acc_sqrt) + soma_b
            ps = psum_pool.tile([NB, 2 * NG], FP32)
            nc.tensor.matmul(ps, sel, acc)
            ps_sb = singles.tile([NB, 2 * NG], FP32)
            nc.scalar.copy(out=ps_sb, in_=ps)
            out_sb = singles.tile([NB, NG], FP32)
            ps_v = ps_sb.rearrange("p (g r) -> p g r", r=2)
            nc.vector.tensor_add(out_sb, ps_v[:, :, 0], ps_v[:, :, 1])
            nc.vector.tensor_scalar_add(out=out_sb, in0=out_sb, scalar1=sbb[0:NB])
            oa = out_d.ap().rearrange("b one -> (b one)")
            nc.sync.dma_start(
                out=bass.AP(tensor=oa.tensor, offset=oa.offset, ap=[[1, NB], [NB, NG]]),
                in_=out_sb,
            )

    nc.compile()
    return nc


_NC_CACHE = None


def _get_program():
    global _NC_CACHE
    if _NC_CACHE is None:
        _NC_CACHE = _build_program()
    return _NC_CACHE


def kernel(x, W, soma_w, soma_b, _trace=False):
    nc = _get_program()
    x = np.ascontiguousarray(x, dtype=np.float32)
    W16 = np.asarray(W, dtype=np.float16)
    soma_w = np.asarray(soma_w, dtype=np.float32)
    soma_b = np.asarray(soma_b, dtype=np.float32)

    w2 = np.ascontiguousarray(np.repeat(W16.reshape(NJ, GF), NB, axis=0))
    sw16 = soma_w.astype(np.float16)
    is_sig = np.arange(D) < CUT
    sw_sig = np.repeat(np.where(is_sig, sw16, 0).reshape(NJ, DJ), NB, axis=0)
    sw_sqrt = np.repeat(np.where(is_sig, 0, sw16).reshape(NJ, DJ), NB, axis=0)
    sw_pair = np.ascontiguousarray(np.concatenate([sw_sig, sw_sqrt], axis=1))
    sel = (np.arange(P)[:, None] % NB == np.arange(NB)[None, :]).astype(np.float32)
    sb = np.full((P, 1), float(soma_b.reshape(-1)[0]), np.float32)
    selsb = np.ascontiguousarray(np.concatenate([sel, sb], axis=1))

    in_maps = [
        {
            "x": x[i * B : (i + 1) * B],
            "W2": w2,
            "SWPAIR": sw_pair,
            "SELSB": selsb,
        }
        for i in range(N_CORES)
    ]
    res = run_bass_kernel_spmd(
        nc, in_maps, core_ids=list(range(N_CORES)), trace=_trace
    )
    out = np.concatenate([r["out"] for r in res.results], axis=0)
    if _trace:
        kernel.last_results = res
    return out.astype(np.float32)


# revision 12
# speedup vs baseline: 1.4482x; 1.0219x over previous
"""Trainium2 Bass kernel for nn_Net_14869176779172 (moe_routing).

Computes, for x[B=1024, D=4096, S=60], W[D, S], soma_w[D], soma_b[1]:
    d[b, j]  = sum_s x[b, j, s] * W[j, s]          (per-dendrite dot)
    r        = relu(d)
    act[b,j] = sigmoid(r)        for j < 1638      (first 40% of dendrites)
             = sqrt(r)           otherwise
    out[b]   = act[b, :] @ soma_w + soma_b         -> [B, 1]

Sharding: pure data-parallel over batch across 8 NeuronCores (128 batch
rows per core); W / soma_w / soma_b replicated.

Per-core layout (v2): partition p = jblk*4 + bidx, where jblk in [0,32)
indexes a block of 128 dendrites and bidx in [0,4) a batch row within a
4-row group.  W is replicated only 4x (host-prepped [128, 7680] fp16
tile, resident in SBUF all kernel), so there is NO per-chunk W broadcast
machinery.  x streams in 32 cast-DMAs (fp32->fp16, 3.9 MB each, 30 KB
contiguous per partition line); DVE does an in-place multiply (2x mode)
+ segmented reduce per group.  Both sigmoid and sqrt are applied
full-width on ScalarE; region selection is folded into two masked
soma-weight tensors (avoids partition-sliced ops, which the BIR
verifier rejects off 32-partition boundaries).  The final
cross-partition soma sum (over jblk) is one matmul with a selector.
"""

import numpy as np

import concourse.bacc as bacc
import concourse.bass as bass
import concourse.tile as tile
from concourse import mybir
from concourse.bass_utils import run_bass_kernel_spmd

# Problem constants (hardcoded per harness contract).
B_TOTAL = 1024
N_CORES = 8
B = B_TOTAL // N_CORES  # 128 batch rows per core
D = 4096
S = 60
CUT = int(D * 0.4)  # 1638: first CUT dendrites use sigmoid, rest sqrt

P = 128  # SBUF partitions
NJ = 32  # dendrite blocks
DJ = D // NJ  # 128 dendrites per block
NB = 4  # batch rows per group (per DMA)
NG = B // NB  # 32 groups
GF = DJ * S  # 7680 elems per partition per group
ROW_F = D * S  # 245760 elems per batch row

FP32 = mybir.dt.float32
FP16 = mybir.dt.float16


def _build_program():
    nc = bacc.Bacc(
        "TRN2",
        target_bir_lowering=False,
        debug=False,
        enable_asserts=False,
        num_devices=N_CORES,
    )

    x_d = nc.dram_tensor("x", [B, D, S], FP32, kind="ExternalInput")
    w2_d = nc.dram_tensor("W2", [P, GF], FP16, kind="ExternalInput")
    # packed small inputs (the NEFF loader rejects >5 input tensors):
    # SWPAIR = [sw_sig | sw_sqrt] fp16; SELSB = [sel | soma_b] fp32
    swpair_d = nc.dram_tensor("SWPAIR", [P, 2 * DJ], FP16, kind="ExternalInput")
    selsb_d = nc.dram_tensor("SELSB", [P, NB + 1], FP32, kind="ExternalInput")
    out_d = nc.dram_tensor("out", [B, 1], FP32, kind="ExternalOutput")

    x_ap = x_d.ap().rearrange("b d s -> b (d s)")  # [128, 245760]

    with tile.TileContext(nc) as tc:
        with (
            tc.tile_pool(name="singles", bufs=1) as singles,
            tc.tile_pool(name="xpool", bufs=6) as xpool,
            tc.tile_pool(name="y2pool", bufs=3) as y2pool,
            tc.tile_pool(name="y3pool", bufs=3) as y3pool,
            tc.tile_pool(name="zpool", bufs=3) as zpool,
            tc.tile_pool(name="zzpool", bufs=3) as zzpool,
            tc.tile_pool(name="scrpool", bufs=2) as scrpool,
            tc.tile_pool(name="psum", bufs=1, space="PSUM") as psum_pool,
        ):
            # ---- resident small tensors (HWDGE loads, overlap with x DMA) ----
            w2 = singles.tile([P, GF], FP16)
            nc.sync.dma_start(out=w2, in_=w2_d.ap())
            sw_pair = singles.tile([P, 2 * DJ], FP16)
            nc.sync.dma_start(out=sw_pair, in_=swpair_d.ap())
            sw_sig = sw_pair[:, 0:DJ]
            sw_sqrt = sw_pair[:, DJ : 2 * DJ]
            selsb = singles.tile([P, NB + 1], FP32)
            nc.sync.dma_start(out=selsb, in_=selsb_d.ap())
            sel = selsb[:, 0:NB]
            sbb = selsb[:, NB : NB + 1]

            # per-(partition, group) soma partial sums: [:, g] sigmoid-region,
            # [:, NG+g] sqrt-region
            acc = singles.tile([P, 2 * NG], FP32)

            for g in range(NG):
                # x group: partition p=(jblk,bidx) reads batch row g*NB+bidx,
                # dendrites [jblk*DJ, (jblk+1)*DJ) -- 30 KB contiguous fp32,
                # cast to fp16 in the SDMA datapath (SWDGE).
                xt = xpool.tile([P, GF], FP16)
                src = bass.AP(
                    tensor=x_ap.tensor,
                    offset=x_ap.offset + g * NB * ROW_F,
                    ap=[[GF, NJ], [ROW_F, NB], [1, GF]],
                )
                nc.gpsimd.dma_start(out=xt, in_=src)

                # y = x * W (in-place, fp16 2x mode)
                nc.vector.tensor_mul(xt, xt, w2)

                # s-reduction 60 -> 30 -> 15 via dense-output tensor_adds
                # (fp16 2x on DVE; strided-OUT ops are slow everywhere, and
                # tensor_reduce has no 2x uop, so reduce only the last 15).
                # Alternate the halving engine per group parity to split the
                # load between DVE and GpSimd.
                xt3 = xt.rearrange("p (c s) -> p c s", s=S)
                y2 = y2pool.tile([P, DJ * 30], FP16)
                y23 = y2.rearrange("p (c s) -> p c s", s=30)
                y3 = y3pool.tile([P, DJ * 15], FP16)
                y33 = y3.rearrange("p (c s) -> p c s", s=15)
                nc.vector.tensor_add(y23, xt3[:, :, 0:30], xt3[:, :, 30:60])
                nc.vector.tensor_add(y33, y23[:, :, 0:15], y23[:, :, 15:30])
                zg = zpool.tile([P, DJ], FP16)
                with nc.allow_low_precision(
                    "fp16 dendrite sums; tol 2e-2, DVE accumulates fp32 internally"
                ):
                    nc.vector.tensor_reduce(
                        out=zg,
                        in_=y33,
                        axis=mybir.AxisListType.X,
                        op=mybir.AluOpType.add,
                    )

                # r = relu(z); zz = [sigmoid(r) | sqrt(r)]   (all ScalarE)
                nc.scalar.activation(
                    out=zg, in_=zg, func=mybir.ActivationFunctionType.Relu
                )
                zz = zzpool.tile([P, 2 * DJ], FP16)
                nc.scalar.activation(
                    out=zz[:, 0:DJ], in_=zg, func=mybir.ActivationFunctionType.Sigmoid
                )
                nc.scalar.activation(
                    out=zz[:, DJ : 2 * DJ],
                    in_=zg,
                    func=mybir.ActivationFunctionType.Sqrt,
                )

                # acc[p, 2g+r] = sum_c zz[r]*sw_pair[r]  (r=0 sig, r=1 sqrt)
                # (tensor_tensor_reduce crashes the runtime; use mul+reduce)
                scr = scrpool.tile([P, 2 * DJ], FP16)
                nc.vector.tensor_mul(scr, zz, sw_pair)
                nc.vector.tensor_reduce(
                    out=acc[:, 2 * g : 2 * g + 2],
                    in_=scr.rearrange("p (r c) -> p r c", r=2),
                    axis=mybir.AxisListType.X,
                    op=mybir.AluOpType.add,
                )

            # ---- final: out[g*NB+i] = sum_jblk (acc_sig + acc_sqrt) + soma_b
            ps = psum_pool.tile([NB, 2 * NG], FP32)
            nc.tensor.matmul(ps, sel, acc)
            ps_sb = singles.tile([NB, 2 * NG], FP32)
            nc.scalar.copy(out=ps_sb, in_=ps)
            out_sb = singles.tile([NB, NG], FP32)
            ps_v = ps_sb.rearrange("p (g r) -> p g r", r=2)
            nc.vector.tensor_add(out_sb, ps_v[:, :, 0], ps_v[:, :, 1])
            nc.vector.tensor_scalar_add(out=out_sb, in0=out_sb, scalar1=sbb[0:NB])
            oa = out_d.ap().rearrange("b one -> (b one)")
            nc.sync.dma_start(
                out=bass.AP(tensor=oa.tensor, offset=oa.offset, ap=[[1, NB], [NB, NG]]),
                in_=out_sb,
            )

    nc.compile()
    return nc


_NC_CACHE = None


def _get_program():
    global _NC_CACHE
    if _NC_CACHE is None:
        _NC_CACHE = _build_program()
    return _NC_CACHE


def kernel(x, W, soma_w, soma_b, _trace=False):
    nc = _get_program()
    x = np.ascontiguousarray(x, dtype=np.float32)
    W16 = np.asarray(W, dtype=np.float16)
    soma_w = np.asarray(soma_w, dtype=np.float32)
    soma_b = np.asarray(soma_b, dtype=np.float32)

    w2 = np.ascontiguousarray(np.repeat(W16.reshape(NJ, GF), NB, axis=0))
    sw16 = soma_w.astype(np.float16)
    is_sig = np.arange(D) < CUT
    sw_sig = np.repeat(np.where(is_sig, sw16, 0).reshape(NJ, DJ), NB, axis=0)
    sw_sqrt = np.repeat(np.where(is_sig, 0, sw16).reshape(NJ, DJ), NB, axis=0)
    sw_pair = np.ascontiguousarray(np.concatenate([sw_sig, sw_sqrt], axis=1))
    sel = (np.arange(P)[:, None] % NB == np.arange(NB)[None, :]).astype(np.float32)
    sb = np.full((P, 1), float(soma_b.reshape(-1)[0]), np.float32)
    selsb = np.ascontiguousarray(np.concatenate([sel, sb], axis=1))

    in_maps = [
        {
            "x": x[i * B : (i + 1) * B],
            "W2": w2,
            "SWPAIR": sw_pair,
            "SELSB": selsb,
        }
        for i in range(N_CORES)
    ]
    res = run_bass_kernel_spmd(
        nc, in_maps, core_ids=list(range(N_CORES)), trace=_trace
    )
    out = np.concatenate([r["out"] for r in res.results], axis=0)
    if _trace:
        kernel.last_results = res
    return out.astype(np.float32)
